# revision 1
# baseline (speedup 1.0000x reference)
"""Trainium2 Bass kernel for nn_GAT (2-layer-mean GAT + NLL loss head).

Contract: kernel(**inputs) takes the FULL inputs of reference.setup_inputs()
and returns the FULL output (the scalar loss, float32), distributing work
across 8 NeuronCores internally.

Strategy (row-shard N=8192 into 8 x 1024 rows, one slab per core):

  For a GAT layer with 0/1 adjacency A: Wh = features @ W,
  src = Wh @ a[:C], dst = Wh @ a[C:], e_ij = leaky_relu(src_i + dst_j),
  attn = row-softmax(where(A>0, e, -9e15)), h' = attn @ Wh.

  Key identity: exp(leaky_relu(x)) = max(exp(x), exp(0.2 x)) (exp is
  monotone, leaky(x) = max(x, 0.2x)), and exp(src_i + dst_j) separates into
  u_i * v_j. With M_ij = [src_i + dst_j > 0] (a 0/1 mask),

    p_ij = A_ij * exp(leaky(e_ij)) = A_ij*M_ij*u_i*v_j
                                   + A_ij*(1-M_ij)*u2_i*v2_j

  so the numerator AND the softmax denominator (appended as a ones-column
  of WhAug = [Wh | 1]) come from two PSUM-accumulated bf16 matmuls per
  adjacency:

    T1[c,i] = sum_j (A o M)[i,j]   * (v  o WhAug)[j,c]
    T2[c,i] = sum_j (A o (1-M))[i,j] * (v2 o WhAug)[j,c]
    h'[i,c] = (T1[c,i] + w_i*T2[c,i]) / (T1[64,i] + w_i*T2[64,i]),
    w_i = exp(-0.8*src_i)   (the u_i factor cancels).

  The device streams the transposed adjacency slabs (int32, cast to bf16
  in-flight by SWDGE DMA), computes M / A o M / A o (1-M) on DVE, and runs
  the matmuls with vW stationary; everything O(N*C) (Wh, exp's, the
  h'-combine, relu-mean, gather, log-softmax, NLL) runs on host numpy.
  No cross-core communication; each core emits T1/T2 for its row slab.
"""

import numpy as np
import ml_dtypes

import concourse.bass as bass
import concourse.bacc as bacc
import concourse.mybir as mybir
import concourse.tile as tile
from concourse.bass_utils import run_bass_kernel_spmd

F32 = mybir.dt.float32
BF16 = mybir.dt.bfloat16
I32 = mybir.dt.int32

ALPHA = 0.2  # LeakyReLU slope of the reference module
N_CORES = 8
N_NODES = 8192
FT = 512
C = 64
CAUG = C + 1
NI = N_NODES // N_CORES  # rows per core


def _build_nc(N=N_NODES, NI_=NI):
    nc = bacc.Bacc("TRN2", target_bir_lowering=False, debug=False)
    JT = N // 128
    NH = NI_ // 512

    adjT1 = nc.dram_tensor("adjT1", [N, NI_], I32, kind="ExternalInput")
    adjT2 = nc.dram_tensor("adjT2", [N, NI_], I32, kind="ExternalInput")
    vW = nc.dram_tensor("vW", [N, CAUG], BF16, kind="ExternalInput")
    v2W = nc.dram_tensor("v2W", [N, CAUG], BF16, kind="ExternalInput")
    srcb = nc.dram_tensor("srcb", [128, NI_], BF16, kind="ExternalInput")
    dstT = nc.dram_tensor("dstT", [128, JT], F32, kind="ExternalInput")
    out = nc.dram_tensor("out", [CAUG, 4 * NI_], F32, kind="ExternalOutput")

    adjT1_t = adjT1.ap().rearrange("(t p) i -> t p i", p=128)
    adjT2_t = adjT2.ap().rearrange("(t p) i -> t p i", p=128)
    vW_t = vW.ap().rearrange("(t p) c -> p t c", p=128)
    v2W_t = v2W.ap().rearrange("(t p) c -> p t c", p=128)

    with tile.TileContext(nc) as tc:
        with (
            tc.tile_pool(name="const", bufs=1) as constp,
            tc.tile_pool(name="io", bufs=1) as iop,
            tc.tile_pool(name="work", bufs=3) as workp,
            tc.tile_pool(name="psum", bufs=1, space="PSUM") as psump,
        ):
            vW_sb = constp.tile([128, JT, CAUG], BF16, tag="vW")
            nc.sync.dma_start(out=vW_sb[:], in_=vW_t)
            v2W_sb = constp.tile([128, JT, CAUG], BF16, tag="v2W")
            nc.sync.dma_start(out=v2W_sb[:], in_=v2W_t)
            srcb_sb = constp.tile([128, NI_], BF16, tag="srcb")
            nc.sync.dma_start(out=srcb_sb[:], in_=srcb.ap())
            dstT_sb = constp.tile([128, JT], F32, tag="dstT")
            nc.sync.dma_start(out=dstT_sb[:], in_=dstT.ap())

            T = [
                psump.tile([CAUG, NI_], F32, tag=f"T{k}", name=f"T{k}")
                for k in range(4)
            ]

            for jt in range(JT):
                adj_bf = []
                for L, adjT_t in ((0, adjT1_t), (1, adjT2_t)):
                    # SWDGE DMA casts int32 -> bf16 in flight (0/1 exact)
                    t = workp.tile([128, NI_], BF16, tag=f"adj{L}", name=f"adj{L}")
                    nc.gpsimd.dma_start(out=t[:], in_=adjT_t[jt])
                    adj_bf.append(t)

                M = workp.tile([128, NI_], BF16, tag="M", name="M")
                nc.vector.tensor_scalar(
                    out=M[:],
                    in0=srcb_sb[:],
                    scalar1=dstT_sb[:, jt : jt + 1],
                    scalar2=0.0,
                    op0=mybir.AluOpType.add,
                    op1=mybir.AluOpType.is_gt,
                )

                for L in (0, 1):
                    A1 = workp.tile([128, NI_], BF16, tag=f"A1_{L}", name=f"A1_{L}")
                    nc.vector.tensor_mul(out=A1[:], in0=adj_bf[L][:], in1=M[:])
                    # A2 = adj * (1 - M) == (M < adj) for 0/1 operands
                    A2 = workp.tile([128, NI_], BF16, tag=f"A2_{L}", name=f"A2_{L}")
                    nc.vector.tensor_tensor(
                        out=A2[:],
                        in0=M[:],
                        in1=adj_bf[L][:],
                        op=mybir.AluOpType.is_lt,
                    )
                    for h in range(NH):
                        sl = slice(512 * h, 512 * (h + 1))
                        nc.tensor.matmul(
                            T[2 * L][:, sl],
                            vW_sb[:, jt, :],
                            A1[:, sl],
                            start=(jt == 0),
                            stop=(jt == JT - 1),
                        )
                        nc.tensor.matmul(
                            T[2 * L + 1][:, sl],
                            v2W_sb[:, jt, :],
                            A2[:, sl],
                            start=(jt == 0),
                            stop=(jt == JT - 1),
                        )

            for k in range(4):
                o = iop.tile([CAUG, NI_], F32, tag=f"out{k}", name=f"out{k}")
                nc.vector.tensor_copy(out=o[:], in_=T[k][:])
                nc.sync.dma_start(out=out.ap()[:, k * NI_ : (k + 1) * NI_], in_=o[:])

    nc.compile()
    return nc


_NC_CACHE = {}


def _get_nc():
    if "nc" not in _NC_CACHE:
        _NC_CACHE["nc"] = _build_nc()
    return _NC_CACHE["nc"]


def kernel(features, W, a, adj1, adj2, labels, idx_train):
    features = np.asarray(features, np.float32)
    W = np.asarray(W, np.float32)
    a = np.asarray(a, np.float32)
    adj1 = np.ascontiguousarray(np.asarray(adj1, np.int32))
    adj2 = np.ascontiguousarray(np.asarray(adj2, np.int32))
    labels = np.asarray(labels, np.int32)
    idx_train = np.asarray(idx_train, np.int32)

    N = features.shape[0]
    assert N == N_NODES and features.shape[1] == FT and W.shape == (FT, C)

    # ---- host prep: everything O(N*C) ----
    Wh = features @ W  # [N, C]
    src = (Wh @ a[:C]).reshape(-1)  # [N]
    dst = (Wh @ a[C:]).reshape(-1)  # [N]
    WhAug = np.concatenate([Wh, np.ones((N, 1), np.float32)], axis=1)
    v = np.exp(dst, dtype=np.float32)
    v2 = np.exp(ALPHA * dst, dtype=np.float32)
    vW_bf = (v[:, None] * WhAug).astype(ml_dtypes.bfloat16)
    v2W_bf = (v2[:, None] * WhAug).astype(ml_dtypes.bfloat16)
    w = np.exp((ALPHA - 1.0) * src, dtype=np.float32)  # u2/u
    dstT = np.ascontiguousarray(dst.reshape(-1, 128).T).astype(np.float32)
    src_bf = src.astype(ml_dtypes.bfloat16)

    # ---- shard: each core gets its transposed adjacency row-slab ----
    in_maps = []
    for c in range(N_CORES):
        i0 = c * NI
        srcb = np.broadcast_to(src_bf[i0 : i0 + NI][None, :], (128, NI))
        in_maps.append(
            dict(
                adjT1=adj1[i0 : i0 + NI, :].T,
                adjT2=adj2[i0 : i0 + NI, :].T,
                vW=vW_bf,
                v2W=v2W_bf,
                srcb=srcb,
                dstT=dstT,
            )
        )

    # ---- run on the 8 NeuronCores ----
    nc = _get_nc()
    res = run_bass_kernel_spmd(nc, in_maps, list(range(N_CORES)))

    # ---- gather/unshard + tiny epilogue on host ----
    h_layers = []  # summed relu(h') over the two adjacencies
    for c in range(N_CORES):
        o = res.results[c]["out"]  # [65, 4*NI]
        i0 = c * NI
        wi = w[i0 : i0 + NI][None, :].astype(np.float64)
        hs = None
        for L in (0, 1):
            T1 = o[:, 2 * L * NI : (2 * L + 1) * NI].astype(np.float64)
            T2 = o[:, (2 * L + 1) * NI : (2 * L + 2) * NI].astype(np.float64)
            num = T1[:C] + wi * T2[:C]  # [C, NI]
            den = T1[C] + wi[0] * T2[C]  # [NI]
            bad = ~(np.abs(den) > 1e-30)
            if bad.any():
                # empty adjacency row: reference softmax is uniform over all j
                den = np.where(bad, 1.0, den)
                num = np.where(bad[None, :], WhAug[:, :C].mean(0)[:, None], num)
            h = np.maximum((num / den).T, 0.0)  # relu(h') [NI, C]
            hs = h if hs is None else hs + h
        h_layers.append(hs * 0.5)
    h_all = np.concatenate(h_layers, axis=0)  # [N, C] float64

    logits = h_all[idx_train]
    m = logits.max(axis=1, keepdims=True)
    logp = logits - (m + np.log(np.exp(logits - m).sum(axis=1, keepdims=True)))
    y = labels[idx_train]
    loss = -logp[np.arange(len(y)), y].mean()
    return np.asarray(loss, dtype=np.float32)
